# revision 11
# baseline (speedup 1.0000x reference)
"""Trainium2 Bass kernel for ChunkCausalDepthwiseConv1d (x-stationary redesign).

Problem: x (16, 512, 4096) f32; per-channel depthwise convs:
  out = chunk_scale * (chunkconv_K31_same_per_256chunk(x) + chunk_b)
        + causalconv_K16(x) + causal_b

Strategy (8 NeuronCores, channel-parallel, 64 ch/core, all batches):
  Per channel, per 256-chunk u: even block E_u = x[256u:256u+128], odd block
  O_u.  Cast the conv as bf16 matmuls where the TRANSPOSED x blocks are the
  STATIONARY operand and small per-channel Toeplitz matrices stream, so psum
  comes out with partitions = (b, q) natural order and NO output transposes
  are needed:
    psE[(b,q), j] = ones*bias_e + E^T ME + O^T MEn[113:] + Oprev^T MEp[:15]
    psO[(b,q), j] = ones*bias_o + O^T MO + E^T  MOp[:15]
  Bias enters as a K=1 matmul (ones row at partition 32*(c//32), bias rows
  DMA'd to the same partition), so every evacuation is a plain psum->sbuf
  copy split between ACT and DVE.
  Input path: per channel 1 DMA x [128=(b,q), 512t] f32 (2 KiB runs), 4 PE
  transposes (fp32r identity, 1.5 cyc/row) -> psum, evac casts to bf16 into
  xtm_e [128,256] / xtm_o [128,400]; xtm_o's extra 144 cols hold a
  (b, q9)-layout duplicate of odd blocks shifted by one chunk (zero col at
  q9=0) to feed the g=0 prev-chunk causal carry with a contiguous-ish AP.
  Output: psum -> onat [128=(b,q), 512] f32 -> 1 DMA (2 KiB runs).
  Weights (~4.8 MiB/core bf16) stream on the SWDGE ring.
"""

import numpy as np

B, C, T = 16, 512, 4096
NCORES = 8
NCH = C // NCORES          # 64 channels per core
PACKW = 304                # cols: ME 128 | MO 128 | MEn 15 | MEp 15 | MOp 15 | pad 3


def _build_mats(causal_w, causal_b, chunk_w, chunk_b, conv_scale):
    """Per-channel moving matrices [t_in, t_out] and bias rows, f32."""
    w1 = np.asarray(causal_w, np.float32)[:, 0, :]   # (C,16)
    w2 = np.asarray(chunk_w, np.float32)[:, 0, :]    # (C,31)
    b1 = np.asarray(causal_b, np.float32)
    b2 = np.asarray(chunk_b, np.float32)
    cs = np.asarray(conv_scale, np.float32)          # (2,C,31)

    scale = np.ones((C, 256), np.float32)
    scale[:, :31] += cs[0]
    scale[:, 225:] += cs[1]

    p = np.arange(128)[:, None]
    j = np.arange(128)[None, :]

    d = p - j + 15
    band2 = (d >= 0) & (d <= 30)
    band1 = (d >= 0) & (d <= 15)
    W2 = w2[:, np.clip(d, 0, 30)] * band2            # (C,128,128)
    W1 = w1[:, np.clip(d, 0, 15)] * band1

    ME = W2 * scale[:, None, :128] + W1
    MO = W2 * scale[:, None, 128:] + W1

    dn = 128 + p - j + 15                            # O_u -> even block
    MEn = (w2[:, np.clip(dn, 0, 30)] * ((dn >= 0) & (dn <= 30))) \
        * scale[:, None, :128]
    dp = p - 128 - j + 15                            # O_{u-1} -> even block
    MEp = w1[:, np.clip(dp, 0, 15)] * ((dp >= 0) & (dp <= 15))
    dq = p - j - 113                                 # E_u -> odd block
    MOp = (w2[:, np.clip(dq, 0, 30)] * ((dq >= 0) & (dq <= 30))) \
        * scale[:, None, 128:] \
        + w1[:, np.clip(dq, 0, 15)] * ((dq >= 0) & (dq <= 15))

    bias_e = scale[:, :128] * b2[:, None] + b1[:, None]
    bias_o = scale[:, 128:] * b2[:, None] + b1[:, None]
    return ME, MO, MEn, MEp, MOp, bias_e, bias_o


def _pack_weights(causal_w, causal_b, chunk_w, chunk_b, conv_scale):
    """(C,128,PACKW) and (C,256) f32 arrays ready for bf16 cast."""
    ME, MO, MEn, MEp, MOp, bias_e, bias_o = _build_mats(
        causal_w, causal_b, chunk_w, chunk_b, conv_scale)
    pack = np.zeros((C, 128, PACKW), np.float32)
    pack[:, :, 0:128] = ME
    pack[:, :, 128:256] = MO
    pack[:, :, 256:271] = MEn[:, :, 113:128]
    pack[:, :, 271:286] = MEp[:, :, 0:15]
    pack[:, :, 286:301] = MOp[:, :, 0:15]
    bias = np.concatenate([bias_e, bias_o], axis=1)  # (C,256)
    return pack, bias


def _bias_rows(bias, nch=NCH):
    """(2, 32*256): row r holds channels 32r..32r+31 of this core's slice."""
    assert bias.shape == (nch, 256) and nch == 64
    return bias.reshape(2, 32 * 256)


def build_nc(nch=NCH, enable_asserts=False, loop_reps=1):
    import concourse.bacc as bacc
    import concourse.mybir as mybir
    import concourse.tile as tile

    fp32 = mybir.dt.float32
    fp32r = mybir.dt.float32r
    bf16 = mybir.dt.bfloat16
    COPY = mybir.ActivationFunctionType.Identity

    nc = bacc.Bacc("TRN2", target_bir_lowering=False, debug=False,
                   enable_asserts=enable_asserts)

    x_d = nc.dram_tensor("x", [B, nch, T], fp32r, kind="ExternalInput").ap()
    w_d = nc.dram_tensor("wpack", [nch, 128, PACKW], bf16, kind="ExternalInput").ap()
    bias_d = nc.dram_tensor("biasrows", [2, 32 * 256], bf16, kind="ExternalInput").ap()
    id_d = nc.dram_tensor("ident", [128, 128], fp32r, kind="ExternalInput").ap()
    o_d = nc.dram_tensor("out", [B, nch, T], fp32, kind="ExternalOutput").ap()

    x_v = x_d.rearrange("b c (q t) -> c b q t", q=8)
    o_v = o_d.rearrange("b c (q t) -> c b q t", q=8)
    WCH = 8
    w_v = w_d.rearrange("(cc c) p w -> cc p c w", cc=nch // WCH)

    with tile.TileContext(nc) as tc:
        with (
            tc.tile_pool(name="wbuf", bufs=1) as wbuf_pool,
            tc.tile_pool(name="const", bufs=1) as const_pool,
            tc.tile_pool(name="xnat", bufs=3) as xnat_pool,
            tc.tile_pool(name="xtme", bufs=2) as xtme_pool,
            tc.tile_pool(name="xtmo", bufs=2) as xtmo_pool,
            tc.tile_pool(name="onat", bufs=3) as onat_pool,
            tc.tile_pool(name="ps_it", bufs=2, space="PSUM") as psit_pool,
            tc.tile_pool(name="ps_conv", bufs=3, space="PSUM") as psconv_pool,
        ):
            wbuf = wbuf_pool.tile([128, nch, PACKW], bf16)
            ident = const_pool.tile([128, 128], fp32r)
            ones = const_pool.tile([128, 128], bf16)
            biasrow = const_pool.tile([128, 32 * 256], bf16)
            nc.vector.memset(ones[:], 1.0)
            nc.gpsimd.dma_start(ident[:], id_d)
            # bias rows to partitions 0 and 32
            nc.gpsimd.dma_start(biasrow[0:1, :], bias_d[0:1, :])
            nc.gpsimd.dma_start(biasrow[32:33, :], bias_d[1:2, :])
            for i in range(nch // WCH):
                nc.gpsimd.dma_start(wbuf[:, i * WCH:(i + 1) * WCH, :], w_v[i])

            import contextlib
            loop_cm = (tc.For_i(0, loop_reps, 1) if loop_reps > 1
                       else contextlib.nullcontext())
            with loop_cm:
              for c in range(nch):
                xnat = xnat_pool.tile([128, 512], fp32r)
                nc.sync.dma_start(xnat[:], x_v[c])

                xtm_e = xtme_pool.tile([128, 256], bf16)
                xtm_o = xtmo_pool.tile([128, 384], bf16)
                dupv = xtm_o[:, 256:384].rearrange("p (b q) -> p b q", b=16)
                nc.vector.memset(dupv[:, :, 0], 0.0)

                for j in range(4):
                    g = j // 2
                    ps = psit_pool.tile([128, 128], fp32r, tag="ps_it")
                    nc.tensor.transpose(
                        ps[:], xnat[:, j * 128:(j + 1) * 128], ident[:])
                    psf = ps[:].bitcast(fp32)
                    if j % 2 == 0:
                        nc.scalar.activation(
                            xtm_e[:, g * 128:(g + 1) * 128], psf, COPY)
                    else:
                        nc.vector.tensor_copy(
                            xtm_o[:, g * 128:(g + 1) * 128], psf)
                        if j == 3:
                            nc.vector.tensor_copy(
                                dupv[:, :, 1:8],
                                psf.rearrange("p (b q) -> p b q", b=16)[:, :, 0:7])

                onat = onat_pool.tile([128, 512], fp32, tag="onat")
                r = c // 32
                p0 = 32 * r
                onesrow = ones[p0:p0 + 1, 0:128]
                bE = biasrow[p0:p0 + 1, (c % 32) * 256:(c % 32) * 256 + 128]
                bO = biasrow[p0:p0 + 1, (c % 32) * 256 + 128:(c % 32) * 256 + 256]
                for g in range(2):
                    psE = psconv_pool.tile([128, 128], fp32, tag="psE")
                    psO = psconv_pool.tile([128, 128], fp32, tag="psO")
                    xE = xtm_e[:, g * 128:(g + 1) * 128]
                    xO = xtm_o[:, g * 128:(g + 1) * 128]
                    xOp = xtm_o[:, 0:128] if g == 1 else xtm_o[:, 256:384]

                    nc.tensor.matmul(psE[:], onesrow, bE,
                                     start=True, stop=False, skip_group_check=True)
                    nc.tensor.matmul(psO[:], onesrow, bO,
                                     start=True, stop=False, skip_group_check=True)
                    nc.tensor.matmul(psE[:], xE, wbuf[:, c, 0:128],
                                     start=False, stop=False, skip_group_check=True)
                    nc.tensor.matmul(psO[:, 0:15], xE, wbuf[:, c, 286:301],
                                     start=False, stop=False, skip_group_check=True)
                    nc.tensor.matmul(psO[:], xO, wbuf[:, c, 128:256],
                                     start=False, stop=True, skip_group_check=True)
                    nc.tensor.matmul(psE[:, 113:128], xO, wbuf[:, c, 256:271],
                                     start=False, stop=False, skip_group_check=True)
                    nc.tensor.matmul(psE[:, 0:15], xOp, wbuf[:, c, 271:286],
                                     start=False, stop=True, skip_group_check=True)

                    nc.scalar.activation(
                        onat[:, g * 256:g * 256 + 128], psE[:], COPY)
                    nc.vector.tensor_copy(
                        onat[:, g * 256 + 128:g * 256 + 256], psO[:])

                nc.scalar.dma_start(o_v[c], onat[:])

    nc.compile()
    return nc


def kernel(x, causal_w, causal_b, chunk_w, chunk_b, conv_scale, chunk_size):
    from concourse.bass_utils import run_bass_kernel_spmd
    import ml_dtypes

    assert int(chunk_size) == 256
    bf = ml_dtypes.bfloat16
    x = np.ascontiguousarray(np.asarray(x, np.float32))
    pack, bias = _pack_weights(causal_w, causal_b, chunk_w, chunk_b, conv_scale)
    ident = np.eye(128, dtype=np.float32)

    nc = build_nc()
    core_ids = list(range(NCORES))
    in_maps = []
    for i in core_ids:
        sl = slice(i * NCH, (i + 1) * NCH)
        in_maps.append({
            "x": np.ascontiguousarray(x[:, sl, :]),
            "wpack": np.ascontiguousarray(pack[sl]).astype(bf),
            "biasrows": _bias_rows(bias[sl]).astype(bf),
            "ident": ident,
        })
    res = run_bass_kernel_spmd(nc, in_maps, core_ids)
    out = np.empty((B, C, T), np.float32)
    for i in core_ids:
        out[:, i * NCH:(i + 1) * NCH, :] = res.results[i]["out"]
    return out
